# revision 30
# baseline (speedup 1.0000x reference)
"""Trainium2 Bass kernel for SimCLR-style contrastive loss (B=8192, D=512).

Symmetric fp8 edition, host-normalized, diag-triangle: the host
pre-normalizes rows to unit length (and precomputes the positive-pair
cosines), so the device kernel is a pure stream: DMA fp8 features ->
PE similarity matmuls -> ACT exp (+row-sum accumulator) -> PE
ones-matmul col-sums -> DMA out.

sim = fn fn^T is symmetric, so each computed entry E_ij =
exp((cos_ij-1)/T + C) credits BOTH row-sum S_i (via the ACT
accumulator) and col-sum S_j (via a PE ones-matmul over the fp8 E
tile).  Each core computes ~8.5/16 of its [1024, 8192] slab:

  diag region, local cols [0, 1024): row-tile t computes only cols
      [t*128, 1024) -- its self 128-block in full (row sums only; both
      mirror entries present) plus the upper strip (row + col sums,
      ragged per-col-block col-sums credit the never-computed lower
      triangle)
  local cols [1024, 4096)  two 1536-wide groups     row + col sums
  local cols [4096, 5120)  antipodal half-block     row + col sums
      row-tiles 0-3 x cols [4096, 4608), row-tiles 4-7 x cols [4608, 5120)

The host hands each core a row-permuted fp8 copy of the normalized
features (transposed, only the 5120 needed columns); for cores 4-7 the
two antipodal half-slabs are swapped so one SPMD program covers every
core.

Per-core outputs: rsums[128, 8] (ACT accum, [partition, row-tile] so
the epilogue DMA is one contiguous 32B line per partition), and
csums[4992] covering local cols [128, 5120).  Host scatter-adds S, then
loss = 1/T - C + mean(ln S) - mean(cos_pos)/T.

E is stored shifted by C=14 so its [0.01, 55] range survives the fp8
cast used by the col-sum matmuls.
"""

import functools
import sys

sys.path.insert(0, "/opt/trn_rl_repo")

import ml_dtypes
import numpy as np

import concourse.bass as bass
import concourse.mybir as mybir
import concourse.tile as tile
from concourse import bacc
from concourse.bass_utils import run_bass_kernel_spmd

B = 8192
D = 512
NCORES = 8
R = B // NCORES  # rows per core
NCOL = B // 2 + R  # columns computed per core
TEMP = 0.07
INV_T = 1.0 / TEMP
C_SHIFT = 14.0

F32 = mybir.dt.float32
BF16 = mybir.dt.bfloat16
FP8 = mybir.dt.float8e4
AF = mybir.ActivationFunctionType
ALU = mybir.AluOpType
DR = mybir.MatmulPerfMode.DoubleRow

KT = D // 128  # 4 k-tiles
KP = KT // 2  # 2 k-tile pairs (DoubleRow)
RT = R // 128  # 8 row tiles
LW = 1024  # DMA piece width
GW = 1536  # main PSUM group width (3 banks)
NMG = 2  # main column groups (cols [1024, 4096))
NSLOT = NMG + 2  # spart accumulation slots (diag + main + antipodal)


@functools.lru_cache(maxsize=None)
def build():
    nc = bacc.Bacc(None, target_bir_lowering=False)
    ftd = nc.dram_tensor("features_t", [D, NCOL], FP8, kind="ExternalInput")
    outr = nc.dram_tensor("rsums", [128, RT], F32, kind="ExternalOutput")
    outc = nc.dram_tensor("csums", [NCOL - 128], F32, kind="ExternalOutput")

    with tile.TileContext(nc) as tc:
        with (
            tc.tile_pool(name="ftp", bufs=1) as ftp,
            tc.tile_pool(name="ep", bufs=1) as epool,
            tc.tile_pool(name="sing", bufs=1) as sing,
            tc.tile_pool(name="cs", bufs=2, space="PSUM") as csp,
            tc.tile_pool(name="mm", bufs=2, space="PSUM") as mmp,
        ):
            ft = [
                ftp.tile([128, 2, NCOL], FP8, tag=f"ft{p}", name=f"ft{p}")
                for p in range(KP)
            ]
            Ediag = epool.tile([128, RT, 1024], FP8)
            E13 = epool.tile([128, RT, NMG * GW], FP8)
            Eap = epool.tile([128, RT, 512], FP8)

            # Constants
            negI = sing.tile([128, 128], BF16)
            nc.gpsimd.memset(negI[:], 0.0)
            nc.gpsimd.affine_select(
                out=negI[:], in_=negI[:], compare_op=ALU.not_equal,
                fill=-1e30, base=0, pattern=[[-1, 128]], channel_multiplier=1,
            )
            eyeb = sing.tile([128, 128], BF16)
            nc.gpsimd.memset(eyeb[:], 0.0)
            nc.gpsimd.affine_select(
                out=eyeb[:], in_=eyeb[:], compare_op=ALU.not_equal,
                fill=1.0, base=0, pattern=[[-1, 128]], channel_multiplier=1,
            )
            ones_dr = sing.tile([128, 2, 16], FP8)
            nc.vector.memset(ones_dr[:], 1.0)
            biasC = sing.tile([128, 1], F32)
            nc.vector.memset(biasC[:], C_SHIFT - INV_T)

            spart = sing.tile([128, RT, NSLOT], F32)
            ssum = sing.tile([128, RT], F32)
            csum_sb = sing.tile([1, NCOL - 128], F32)

            # Loads issue mostly from the sync queue: the ACT engine (the
            # critical resource) must not spend ~620ns slots on DMA
            # issuance mid-kernel, and GpSimd must never touch the DGE (its
            # end-of-kernel dge drain costs ~5us once any gpsimd DMA
            # descriptor exists).  The first two pieces split their issue
            # across sync+scalar (scalar is idle until the first exp, ~5
            # DMAs later) so the first diag matmuls start ~5us sooner.
            def load_cols(c0, c1, engines=(nc.sync,)):
                ls = slice(c0, c1)
                for k in range(KT):
                    engines[k % len(engines)].dma_start(
                        out=ft[k // 2][:, k % 2, ls],
                        in_=ftd[k * 128 : (k + 1) * 128, ls],
                    )

            def mm_row_block(ps, t, col0, width):
                """Accumulate sim[t-block rows, col0:col0+width] into psum.

                Splits into <=512 chunks aligned to the psum tile start so
                no chunk crosses a PSUM bank boundary.
                """
                chunks = []
                c = 0
                while c < width:
                    w = min(512, width - c)
                    chunks.append((c, w))
                    c += w
                for p in range(KP):
                    for c, w in chunks:
                        nc.tensor.matmul(
                            ps[:, c : c + w],
                            ft[p][:, :, t * 128 : (t + 1) * 128],
                            ft[p][:, :, col0 + c : col0 + c + w],
                            start=(p == 0),
                            stop=(p == KP - 1),
                            perf_mode=DR,
                        )

            def colsum(lo, width, rhs_tile, rhs_lo, upairs, *, tag):
                """Column sums of E over row-tile pairs -> stage in SBUF.

                lo: local column (>= 128) of the first summed column;
                rhs_tile[:, 2u:2u+2, rhs_lo:rhs_lo+width] are the E slabs.
                """
                cps = csp.tile([16, 512], F32, name=f"cs{tag}", tag="cs")
                for i, u in enumerate(upairs):
                    nc.tensor.matmul(
                        cps[:, 0:width],
                        ones_dr[:],
                        rhs_tile[:, 2 * u : 2 * u + 2, rhs_lo : rhs_lo + width],
                        start=(i == 0),
                        stop=(i == len(upairs) - 1),
                        perf_mode=DR,
                    )
                nc.vector.tensor_copy(
                    out=csum_sb[0:1, lo - 128 : lo - 128 + width],
                    in_=cps[0:1, 0:width],
                )

            def colsum_diag():
                """Ragged col sums for the diag region (cols [128, 1024)).

                Col-block b is credited by row-tiles t < b: floor(b/2) DR
                pair passes plus one single-row pass when b is odd.  Each
                col-block is its own accumulation group: a later start=True
                issued after an accumulate into a neighboring region of the
                same psum tile corrupts it on hw, so no range batching.
                """
                for half, brange in ((0, range(1, 4)), (1, range(4, 8))):
                    cps = csp.tile([16, 512], F32, name=f"csd{half}", tag="cs")
                    base = 128 if half == 0 else 512
                    for b in brange:
                        col0 = b * 128
                        off = col0 - base
                        npair = b // 2
                        for u in range(npair):
                            nc.tensor.matmul(
                                cps[:, off : off + 128],
                                ones_dr[:],
                                Ediag[:, 2 * u : 2 * u + 2, col0 : col0 + 128],
                                start=(u == 0),
                                stop=(u == npair - 1 and b % 2 == 0),
                                perf_mode=DR,
                            )
                        if b % 2 == 1:
                            nc.tensor.matmul(
                                cps[:, off : off + 128],
                                ones_dr[:, 0, :],
                                Ediag[:, b - 1, col0 : col0 + 128],
                                start=(npair == 0),
                                stop=True,
                            )
                    w = 384 if half == 0 else 512
                    nc.vector.tensor_copy(
                        out=csum_sb[0:1, base - 128 : base - 128 + w],
                        in_=cps[0:1, 0:w],
                    )

            # Piece [512, 1024) first: the diag loop starts at t=4 whose
            # stationary AND moving columns live entirely in it, so the PE
            # starts ~5us sooner than waiting for cols [0, 1024).
            load_cols(512, 1024, engines=(nc.sync, nc.scalar))

            # Exp table load (1283ns) slots between the scalar queue's two
            # DMA issue pairs: after the piece gating the first matmul,
            # before the piece that is only needed ~2us later.  The first
            # activation then waits on neither.
            _tl = mybir.InstLoadActFuncSet(
                name=nc.get_next_instruction_name(),
                act_func_set_id=6,  # natural_log_exp_and_others
                ins=[],
                outs=[],
            )
            nc.scalar.add_instruction(_tl)

            load_cols(0, 512, engines=(nc.sync, nc.scalar))
            # Remaining pieces aligned to the main group boundaries so each
            # group's first matmul never waits on a split DMA piece.
            load_cols(1024, 1024 + GW)
            load_cols(1024 + GW, 1024 + 2 * GW)
            load_cols(4096, NCOL)

            # Diag region: row-tile t computes cols [t*128, 1024).
            # t=4..7 run first (they only need cols [512, 1024)).
            for t in (4, 5, 6, 7, 0, 1, 2, 3):
                W = 1024 - t * 128
                ps = mmp.tile([128, GW], F32, tag="mm", name=f"dps{t}")
                mm_row_block(ps, t, t * 128, W)
                # Kill the self-pair diagonal (at ps[:, 0:128]).  (Letting
                # it saturate in the fp8 store and subtracting host-side
                # does NOT work: the cast yields inf/nan, not 448.)
                nc.tensor.matmul(
                    ps[:, 0:128], eyeb[:], negI[:],
                    start=False, stop=True, skip_group_check=True,
                )
                nc.scalar.activation(
                    out=Ediag[:, t, t * 128 : t * 128 + W],
                    in_=ps[:, 0:W], func=AF.Exp,
                    scale=INV_T, bias=biasC[:],
                )
                # Row-sum on DVE (idle this early) instead of the ACT
                # accumulator: saves the fixed 187ns accumulator-read per
                # activation on the critical ACT queue.
                nc.vector.tensor_reduce(
                    out=spart[:, t, 0:1],
                    in_=Ediag[:, t, t * 128 : t * 128 + W],
                    axis=mybir.AxisListType.X,
                    op=ALU.add,
                )

            def antip_tile(u):
                """Antipodal tile: row-tile u x 512 cols at [4096, 4608)
                (u<4) or [4608, 5120) (u>=4)."""
                aoff = 4096 + (0 if u < 4 else 512)
                ps = mmp.tile([128, GW], F32, tag="mm", name=f"aps{u}")
                mm_row_block(ps, u, aoff, 512)
                nc.scalar.activation(
                    out=Eap[:, u, :], in_=ps[:, 0:512], func=AF.Exp,
                    scale=INV_T, bias=biasC[:],
                    accum_out=spart[:, u, NSLOT - 1 : NSLOT],
                )

            # Main groups: cols [1024 + g*GW, 1024 + (g+1)*GW).  The main
            # phases are ACT-heavy (~1.66us/tile vs ~1.3us PE) while the
            # antipodal tiles are PE-cheap (~0.5us), so the antip tiles
            # interleave into the main loops (first at g0/t=3, when their
            # DMA piece has landed) instead of forming a PE-bound tail.
            for g in range(NMG):
                for t in range(RT):
                    ps = mmp.tile([128, GW], F32, tag="mm")
                    mm_row_block(ps, t, 1024 + g * GW, GW)
                    nc.scalar.activation(
                        out=E13[:, t, g * GW : (g + 1) * GW],
                        in_=ps[:], func=AF.Exp,
                        scale=INV_T, bias=biasC[:],
                        accum_out=spart[:, t, 1 + g : 2 + g],
                    )
                    if g == 0 and t in (3, 5, 7):
                        antip_tile((t - 3) // 2)  # u = 0, 1, 2
                    if g == 1 and t in (1, 3, 5, 7):
                        antip_tile(3 + (t - 1) // 2)  # u = 3, 4, 5, 6
                    if g == 1 and t == 2:
                        colsum(4096, 512, Eap, 0, [0, 1], tag="apA")
                    if g == 1 and t in (4, 5, 6):
                        # g0 colsums in g1's loop shadow (their deps, the
                        # g0 acts, cleared while g1 started computing).
                        s3 = t - 4
                        colsum(1024 + s3 * 512, 512, E13, s3 * 512,
                               range(4), tag=f"g0s{s3}")
                if g == 0:
                    colsum_diag()
                else:
                    antip_tile(7)
                    for s3 in range(3):
                        colsum(1024 + GW + s3 * 512, 512, E13, GW + s3 * 512,
                               range(4), tag=f"g1s{s3}")

            colsum(4608, 512, Eap, 0, [2, 3], tag="apB")

            # Row-sum totals: all spart slots are complete once the last
            # main act and antip accum have run.
            for t in range(RT):
                nc.vector.tensor_reduce(
                    out=ssum[:, t : t + 1],
                    in_=spart[:, t, :],
                    axis=mybir.AxisListType.X,
                    op=ALU.add,
                )

            # Epilogue DMAs.
            nc.sync.dma_start(out=outr[:, :], in_=ssum[:, :RT])
            nc.sync.dma_start(out=outc[:], in_=csum_sb[0:1, :])

    nc.finalize()
    return nc


def core_perm(c):
    """Global row index for each local column of core c."""
    perm = (np.arange(B) + c * R) % B
    if c >= NCORES // 2:
        tmp = perm[4096:4608].copy()
        perm[4096:4608] = perm[4608:5120]
        perm[4608:5120] = tmp
    return perm[:NCOL]


def _normalize(feats):
    f = np.asarray(feats, dtype=np.float32)
    n = np.linalg.norm(f, axis=1, keepdims=True)
    return f / np.maximum(n, 1e-12)


def make_in_map(feats, c, fn=None):
    if fn is None:
        fn = _normalize(feats)
    perm = core_perm(c)
    return {
        "features_t": np.ascontiguousarray(fn[perm].T).astype(
            ml_dtypes.float8_e4m3
        )
    }


def run(features, **kwargs):
    """Run the SPMD kernel; returns (y[b] fp32 per-row losses-ish, results).

    y_i = ln(S_i) - C_SHIFT - pos_i/T, so loss = 1/T + mean(y).
    """
    nc = build()
    feats = np.ascontiguousarray(np.asarray(features, dtype=np.float32))
    fn = _normalize(feats)
    in_maps = [make_in_map(feats, c, fn=fn) for c in range(NCORES)]
    res = run_bass_kernel_spmd(nc, in_maps, core_ids=list(range(NCORES)), **kwargs)
    S = np.zeros(B, dtype=np.float64)
    for c in range(NCORES):
        perm = core_perm(c)
        # rsums arrives [partition, row-tile]; local row r = t*128 + p.
        rs = res.results[c]["rsums"].astype(np.float64).T.ravel()
        S[perm[:R]] += rs
        # csums covers local cols [128, 5120).
        S[perm[128:NCOL]] += res.results[c]["csums"].astype(np.float64)
    fn64 = fn.astype(np.float64)
    pos = np.sum(fn64 * np.roll(fn64, B // 2, axis=0), axis=1)  # cos(i, i+B/2)
    y = np.log(S) - C_SHIFT - INV_T * pos
    return y.astype(np.float32), res


def kernel(features):
    y, _ = run(features)
    loss = INV_T + float(np.mean(y.astype(np.float64)))
    return np.float32(loss)


# revision 31
# speedup vs baseline: 1.0916x; 1.0916x over previous
"""Trainium2 Bass kernel for SimCLR-style contrastive loss (B=8192, D=512).

Symmetric fp8 edition, host-normalized, diag-triangle: the host
pre-normalizes rows to unit length (and precomputes the positive-pair
cosines), so the device kernel is a pure stream: DMA fp8 features ->
PE similarity matmuls -> ACT exp (+row-sum accumulator) -> PE
ones-matmul col-sums -> DMA out.

sim = fn fn^T is symmetric, so each computed entry E_ij =
exp((cos_ij-1)/T + C) credits BOTH row-sum S_i (via the ACT
accumulator) and col-sum S_j (via a PE ones-matmul over the fp8 E
tile).  Each core computes ~8.5/16 of its [1024, 8192] slab:

  diag region, local cols [0, 1024): row-tile t computes only cols
      [t*128, 1024) -- its self 128-block in full (row sums only; both
      mirror entries present) plus the upper strip (row + col sums,
      ragged per-col-block col-sums credit the never-computed lower
      triangle)
  local cols [1024, 4096)  two 1536-wide groups     row + col sums
  local cols [4096, 5120)  antipodal half-block     row + col sums
      row-tiles 0-3 x cols [4096, 4608), row-tiles 4-7 x cols [4608, 5120)

The host hands each core a row-permuted fp8 copy of the normalized
features (transposed, only the 5120 needed columns); for cores 4-7 the
two antipodal half-slabs are swapped so one SPMD program covers every
core.

Per-core outputs: rsums[128, 8] (ACT accum, [partition, row-tile] so
the epilogue DMA is one contiguous 32B line per partition), and
csums[4992] covering local cols [128, 5120).  Host scatter-adds S, then
loss = 1/T - C + mean(ln S) - mean(cos_pos)/T.

E is stored shifted by C=14 so its [0.01, 55] range survives the fp8
cast used by the col-sum matmuls.
"""

import functools
import sys

sys.path.insert(0, "/opt/trn_rl_repo")

import ml_dtypes
import numpy as np

import concourse.bass as bass
import concourse.mybir as mybir
import concourse.tile as tile
from concourse import bacc
from concourse.bass_utils import run_bass_kernel_spmd

B = 8192
D = 512
NCORES = 8
R = B // NCORES  # rows per core
NCOL = B // 2 + R  # columns computed per core
TEMP = 0.07
INV_T = 1.0 / TEMP
C_SHIFT = 14.0

F32 = mybir.dt.float32
BF16 = mybir.dt.bfloat16
FP8 = mybir.dt.float8e4
AF = mybir.ActivationFunctionType
ALU = mybir.AluOpType
DR = mybir.MatmulPerfMode.DoubleRow

KT = D // 128  # 4 k-tiles
KP = KT // 2  # 2 k-tile pairs (DoubleRow)
RT = R // 128  # 8 row tiles
LW = 1024  # DMA piece width
GW = 1536  # main PSUM group width (3 banks)
NMG = 2  # main column groups (cols [1024, 4096))
NSLOT = NMG + 2  # spart accumulation slots (diag + main + antipodal)


@functools.lru_cache(maxsize=None)
def build():
    nc = bacc.Bacc(None, target_bir_lowering=False)
    ftd = nc.dram_tensor("features_t", [D, NCOL], FP8, kind="ExternalInput")
    outr = nc.dram_tensor("rsums", [128, RT], F32, kind="ExternalOutput")
    outc = nc.dram_tensor("csums", [NCOL - 128], F32, kind="ExternalOutput")

    with tile.TileContext(nc) as tc:
        with (
            tc.tile_pool(name="ftp", bufs=1) as ftp,
            tc.tile_pool(name="ep", bufs=1) as epool,
            tc.tile_pool(name="sing", bufs=1) as sing,
            tc.tile_pool(name="cs", bufs=2, space="PSUM") as csp,
            tc.tile_pool(name="mm", bufs=2, space="PSUM") as mmp,
        ):
            ft = [
                ftp.tile([128, 2, NCOL], FP8, tag=f"ft{p}", name=f"ft{p}")
                for p in range(KP)
            ]
            Ediag = epool.tile([128, RT, 1024], FP8)
            E13 = epool.tile([128, RT, NMG * GW], FP8)
            Eap = epool.tile([128, RT, 512], FP8)

            # Constants
            negI = sing.tile([128, 128], BF16)
            nc.gpsimd.memset(negI[:], 0.0)
            nc.gpsimd.affine_select(
                out=negI[:], in_=negI[:], compare_op=ALU.not_equal,
                fill=-1e30, base=0, pattern=[[-1, 128]], channel_multiplier=1,
            )
            eyeb = sing.tile([128, 128], BF16)
            nc.gpsimd.memset(eyeb[:], 0.0)
            nc.gpsimd.affine_select(
                out=eyeb[:], in_=eyeb[:], compare_op=ALU.not_equal,
                fill=1.0, base=0, pattern=[[-1, 128]], channel_multiplier=1,
            )
            ones_dr = sing.tile([128, 2, 16], FP8)
            nc.vector.memset(ones_dr[:], 1.0)
            biasC = sing.tile([128, 1], F32)
            nc.vector.memset(biasC[:], C_SHIFT - INV_T)

            spart = sing.tile([128, RT, NSLOT], F32)
            ssum = sing.tile([128, RT], F32)
            csum_sb = sing.tile([1, NCOL - 128], F32)

            # Loads issue mostly from the sync queue: the ACT engine (the
            # critical resource) must not spend ~620ns slots on DMA
            # issuance mid-kernel, and GpSimd must never touch the DGE (its
            # end-of-kernel dge drain costs ~5us once any gpsimd DMA
            # descriptor exists).  The first two pieces split their issue
            # across sync+scalar (scalar is idle until the first exp, ~5
            # DMAs later) so the first diag matmuls start ~5us sooner.
            def load_cols(c0, c1, engines=(nc.sync,)):
                ls = slice(c0, c1)
                for k in range(KT):
                    engines[k % len(engines)].dma_start(
                        out=ft[k // 2][:, k % 2, ls],
                        in_=ftd[k * 128 : (k + 1) * 128, ls],
                    )

            def mm_row_block(ps, t, col0, width):
                """Accumulate sim[t-block rows, col0:col0+width] into psum.

                Splits into <=512 chunks aligned to the psum tile start so
                no chunk crosses a PSUM bank boundary.
                """
                chunks = []
                c = 0
                while c < width:
                    w = min(512, width - c)
                    chunks.append((c, w))
                    c += w
                for p in range(KP):
                    for c, w in chunks:
                        nc.tensor.matmul(
                            ps[:, c : c + w],
                            ft[p][:, :, t * 128 : (t + 1) * 128],
                            ft[p][:, :, col0 + c : col0 + c + w],
                            start=(p == 0),
                            stop=(p == KP - 1),
                            perf_mode=DR,
                        )

            def colsum(lo, width, rhs_tile, rhs_lo, upairs, *, tag):
                """Column sums of E over row-tile pairs -> stage in SBUF.

                lo: local column (>= 128) of the first summed column;
                rhs_tile[:, 2u:2u+2, rhs_lo:rhs_lo+width] are the E slabs.
                """
                cps = csp.tile([16, 512], F32, name=f"cs{tag}", tag="cs")
                for i, u in enumerate(upairs):
                    nc.tensor.matmul(
                        cps[:, 0:width],
                        ones_dr[:],
                        rhs_tile[:, 2 * u : 2 * u + 2, rhs_lo : rhs_lo + width],
                        start=(i == 0),
                        stop=(i == len(upairs) - 1),
                        perf_mode=DR,
                    )
                nc.vector.tensor_copy(
                    out=csum_sb[0:1, lo - 128 : lo - 128 + width],
                    in_=cps[0:1, 0:width],
                )

            def colsum_diag():
                """Ragged col sums for the diag region (cols [128, 1024)).

                Col-block b is credited by row-tiles t < b: floor(b/2) DR
                pair passes plus one single-row pass when b is odd.  Each
                col-block is its own accumulation group: a later start=True
                issued after an accumulate into a neighboring region of the
                same psum tile corrupts it on hw, so no range batching.
                """
                for half, brange in ((0, range(1, 4)), (1, range(4, 8))):
                    cps = csp.tile([16, 512], F32, name=f"csd{half}", tag="cs")
                    base = 128 if half == 0 else 512
                    for b in brange:
                        col0 = b * 128
                        off = col0 - base
                        npair = b // 2
                        for u in range(npair):
                            nc.tensor.matmul(
                                cps[:, off : off + 128],
                                ones_dr[:],
                                Ediag[:, 2 * u : 2 * u + 2, col0 : col0 + 128],
                                start=(u == 0),
                                stop=(u == npair - 1 and b % 2 == 0),
                                perf_mode=DR,
                            )
                        if b % 2 == 1:
                            nc.tensor.matmul(
                                cps[:, off : off + 128],
                                ones_dr[:, 0, :],
                                Ediag[:, b - 1, col0 : col0 + 128],
                                start=(npair == 0),
                                stop=True,
                            )
                    w = 384 if half == 0 else 512
                    nc.vector.tensor_copy(
                        out=csum_sb[0:1, base - 128 : base - 128 + w],
                        in_=cps[0:1, 0:w],
                    )

            # Piece [512, 1024) first: the diag loop starts at t=4 whose
            # stationary AND moving columns live entirely in it, so the PE
            # starts ~5us sooner than waiting for cols [0, 1024).
            load_cols(512, 1024, engines=(nc.sync, nc.scalar))

            # Exp table load (1283ns) slots between the scalar queue's two
            # DMA issue pairs: after the piece gating the first matmul,
            # before the piece that is only needed ~2us later.  The first
            # activation then waits on neither.
            _tl = mybir.InstLoadActFuncSet(
                name=nc.get_next_instruction_name(),
                act_func_set_id=6,  # natural_log_exp_and_others
                ins=[],
                outs=[],
            )
            nc.scalar.add_instruction(_tl)

            load_cols(0, 512, engines=(nc.sync, nc.scalar))
            # Remaining pieces aligned to the main group boundaries so each
            # group's first matmul never waits on a split DMA piece.
            load_cols(1024, 1024 + GW)
            load_cols(1024 + GW, 1024 + 2 * GW)
            load_cols(4096, NCOL)

            # Diag region: row-tile t computes cols [t*128, 1024).
            # t=4..7 run first (they only need cols [512, 1024)).
            for t in (4, 5, 6, 7, 0, 1, 2, 3):
                W = 1024 - t * 128
                ps = mmp.tile([128, GW], F32, tag="mm", name=f"dps{t}")
                mm_row_block(ps, t, t * 128, W)
                # Kill the self-pair diagonal (at ps[:, 0:128]).  (Letting
                # it saturate in the fp8 store and subtracting host-side
                # does NOT work: the cast yields inf/nan, not 448.)
                nc.tensor.matmul(
                    ps[:, 0:128], eyeb[:], negI[:],
                    start=False, stop=True, skip_group_check=True,
                )
                nc.scalar.activation(
                    out=Ediag[:, t, t * 128 : t * 128 + W],
                    in_=ps[:, 0:W], func=AF.Exp,
                    scale=INV_T, bias=biasC[:],
                )
                # Row-sum on DVE (idle this early) instead of the ACT
                # accumulator: saves the fixed 187ns accumulator-read per
                # activation on the critical ACT queue.
                nc.vector.tensor_reduce(
                    out=spart[:, t, 0:1],
                    in_=Ediag[:, t, t * 128 : t * 128 + W],
                    axis=mybir.AxisListType.X,
                    op=ALU.add,
                )

            # Main groups: cols [1024 + g*GW, 1024 + (g+1)*GW).
            # (Interleaving the antipodal tiles into these loops was tried
            # and REGRESSED ~8us: the 2-buffer PSUM rotation thrashes when
            # a short antip tile slots between 1536-wide main tiles, and
            # the extra PE gaps drop the PE clock to low p-state.)
            for g in range(NMG):
                for t in range(RT):
                    ps = mmp.tile([128, GW], F32, tag="mm")
                    mm_row_block(ps, t, 1024 + g * GW, GW)
                    nc.scalar.activation(
                        out=E13[:, t, g * GW : (g + 1) * GW],
                        in_=ps[:], func=AF.Exp,
                        scale=INV_T, bias=biasC[:],
                        accum_out=spart[:, t, 1 + g : 2 + g],
                    )
                if g == 0:
                    colsum_diag()
                else:
                    for s3 in range(3):
                        colsum(
                            1024 + s3 * 512, 512, E13, s3 * 512,
                            range(4), tag=f"g0s{s3}",
                        )

            # Antipodal half-block: row-tiles 0-3 x [4096, 4608),
            # row-tiles 4-7 x [4608, 5120).  The group-1 and apA colsums
            # interleave into the loop so only apB's colsum trails the last
            # activation.
            for t in range(RT):
                aoff = 4096 + (0 if t < 4 else 512)
                ps = mmp.tile([128, GW], F32, tag="mm", name=f"aps{t}")
                mm_row_block(ps, t, aoff, 512)
                nc.scalar.activation(
                    out=Eap[:, t, :], in_=ps[:, 0:512], func=AF.Exp,
                    scale=INV_T, bias=biasC[:],
                    accum_out=spart[:, t, NSLOT - 1 : NSLOT],
                )
                # This row-tile's S partial is complete -> reduce now so the
                # final rsums DMA fires right after the last antip exp.
                nc.vector.tensor_reduce(
                    out=ssum[:, t : t + 1],
                    in_=spart[:, t, :],
                    axis=mybir.AxisListType.X,
                    op=ALU.add,
                )
                if t == 0:
                    for s3 in range(3):
                        colsum(1024 + GW + s3 * 512, 512, E13, GW + s3 * 512,
                               range(4), tag=f"g1s{s3}")
                if t == 4:
                    colsum(4096, 512, Eap, 0, [0, 1], tag="apA")
                    # Everything below local col 4608 is staged: ship it
                    # while the t>=4 antip tiles finish.
                    nc.sync.dma_start(
                        out=outc[0 : 4608 - 128], in_=csum_sb[0:1, 0 : 4608 - 128]
                    )
            colsum(4608, 512, Eap, 0, [2, 3], tag="apB")

            # Epilogue: only rsums and the 2KB apB slice remain.
            nc.sync.dma_start(out=outr[:, :], in_=ssum[:, :RT])
            nc.sync.dma_start(
                out=outc[4608 - 128 :], in_=csum_sb[0:1, 4608 - 128 :]
            )

    nc.finalize()
    return nc


def core_perm(c):
    """Global row index for each local column of core c."""
    perm = (np.arange(B) + c * R) % B
    if c >= NCORES // 2:
        tmp = perm[4096:4608].copy()
        perm[4096:4608] = perm[4608:5120]
        perm[4608:5120] = tmp
    return perm[:NCOL]


def _normalize(feats):
    f = np.asarray(feats, dtype=np.float32)
    n = np.linalg.norm(f, axis=1, keepdims=True)
    return f / np.maximum(n, 1e-12)


def make_in_map(feats, c, fn=None):
    if fn is None:
        fn = _normalize(feats)
    perm = core_perm(c)
    return {
        "features_t": np.ascontiguousarray(fn[perm].T).astype(
            ml_dtypes.float8_e4m3
        )
    }


def run(features, **kwargs):
    """Run the SPMD kernel; returns (y[b] fp32 per-row losses-ish, results).

    y_i = ln(S_i) - C_SHIFT - pos_i/T, so loss = 1/T + mean(y).
    """
    nc = build()
    feats = np.ascontiguousarray(np.asarray(features, dtype=np.float32))
    fn = _normalize(feats)
    in_maps = [make_in_map(feats, c, fn=fn) for c in range(NCORES)]
    res = run_bass_kernel_spmd(nc, in_maps, core_ids=list(range(NCORES)), **kwargs)
    S = np.zeros(B, dtype=np.float64)
    for c in range(NCORES):
        perm = core_perm(c)
        # rsums arrives [partition, row-tile]; local row r = t*128 + p.
        rs = res.results[c]["rsums"].astype(np.float64).T.ravel()
        S[perm[:R]] += rs
        # csums covers local cols [128, 5120).
        S[perm[128:NCOL]] += res.results[c]["csums"].astype(np.float64)
    fn64 = fn.astype(np.float64)
    pos = np.sum(fn64 * np.roll(fn64, B // 2, axis=0), axis=1)  # cos(i, i+B/2)
    y = np.log(S) - C_SHIFT - INV_T * pos
    return y.astype(np.float32), res


def kernel(features):
    y, _ = run(features)
    loss = INV_T + float(np.mean(y.astype(np.float64)))
    return np.float32(loss)
